# revision 8
# baseline (speedup 1.0000x reference)
"""Causal attention (B=1, H=16, S=4096, D=64, f32) on 8 trn2 NeuronCores.

Strategy (head-parallel, 2 heads per core):
  - Host pre-transposes Q, K per head to [D, S] (d-major) so the QK^T
    matmul needs no on-device transpose: S^T[k, q] = sum_d K^T[d,k] Q^T[d,q].
  - S^T layout keeps k on PSUM partitions and q on the free axis, so
    exp(S^T) -> P^T lands in SBUF exactly as the rhs of the PV matmul:
    O^T[d, q] = sum_k V[k, d] P^T[k, q], accumulated over k-tiles in PSUM.
  - l[q] = sum_k exp is obtained for free by appending a ones column to V
    (column 64 of the PV matmul output). Host epilogue: O = (O^T[:64]/l).T.

Hybrid exp across TWO engines (the single biggest win over v1):
  Host scales q,k by sqrt(2^20/ln2) so the QK^T matmul directly produces
  y = 2^23/ln2 * (q.k/8) in PSUM. Then either engine can finish softmax's
  exp:
   - ScalarE: activation(Exp, scale=ln2/2^23) recovers exact exp(q.k/8).
   - VectorE: tensor_scalar add of B = 127*2^23 - C and convert to int32;
     the int32 bit pattern REINTERPRETED as float32 is Schraudolph's
     fast-exp approximation (~3% sawtooth error, which softmax
     normalization cancels to ~2e-3 in the final output). One 1x DVE op
     per element, about the same throughput as ScalarE's exp.
  Off-diagonal chunks are routed greedily to balance the two engines;
  diagonal chunks always take the exact ScalarE path (short softmax rows
  are most error-sensitive).

  Causality: k-tiles strictly below the diagonal are skipped entirely; the
  4 diagonal k-tiles per q-block keep only q-columns >= 128*t (QK and PV
  run with a reduced moving dim; PSUM bank-clear zeroes the rest), and the
  single 128x128 triangular corner is masked by an in-place VectorE
  multiply with a constant 0/1 tile.

Matmul dtypes: fp16 throughout (q,k pre-scaled on host; V cast host-side;
P^T is either ScalarE fp16 exp output or the int16 Schraudolph bit
pattern viewed as fp16; the exp trick uses fp16's 2^10 mantissa scale). QK^T matmuls go two-at-a-time in disjoint PE row
groups (rows 0-63 / 64-127 hold identical data).
"""

import os
import sys
import numpy as np

sys.path.insert(0, "/opt/trn_rl_repo")

import concourse.bass as bass
import concourse.mybir as mybir
from concourse.tile import TileContext

B, H, S, D = 1, 16, 4096, 64

PROGRAM_META: dict[str, str] = {}   # instruction name -> kind (for tracing)


def _note(inst, kind):
    try:
        inst.annotate(kind)
        PROGRAM_META[str(inst.ins.name)] = kind
    except Exception:
        pass

N_CORES = 8
H_PER = H // N_CORES          # heads per core
QB = 512                      # q-block (matmul moving dim / PSUM bank)
KT = 128                      # k-tile (contraction tile for PV matmul)
NQB = S // QB                 # 8
NKT = S // KT                 # 32
VW = D + 1                    # V columns + ones column for the l sum

F32 = mybir.dt.float32
F32R = mybir.dt.float32r
F16 = mybir.dt.float16
I16 = mybir.dt.int16

EXP_A = float(2.0 ** 10) / float(np.log(2.0))   # y = EXP_A * (q.k/8)
SCHRAUDOLPH_C = 55.0
SCHRAUDOLPH_B = 15.0 * 2.0 ** 10 - SCHRAUDOLPH_C
QK_SIDE_SCALE = float(np.sqrt(EXP_A / 8.0))     # folded into q AND k


def round_fp32r(x: np.ndarray) -> np.ndarray:
    """fp32 -> fp32r: round-half-to-even at mantissa bit 12 (keep 11 bits)."""
    u = np.ascontiguousarray(x, dtype=np.float32).view(np.uint32)
    r = (u + np.uint32(0x7FF) + ((u >> np.uint32(12)) & np.uint32(1))) & np.uint32(
        0xFFFFF000
    )
    return r.view(np.float32)


def build_program() -> bass.Bass:
    nc = bass.Bass()
    # qk rows 0-63 and 64-127 hold identical qT|kT data: the duplicate lets
    # two QK^T matmuls run concurrently in disjoint PE row groups
    qk_d = nc.declare_dram_parameter("qk", [H_PER, 2 * D, 2 * S], F16, isOutput=False)
    va_d = nc.declare_dram_parameter("va", [H_PER, 128, NKT * VW], F16, isOutput=False)
    mk_d = nc.declare_dram_parameter("mk", [128, KT], F16, isOutput=False)
    oT_d = nc.declare_dram_parameter("outT", [H_PER, VW, S], F32, isOutput=True)

    with TileContext(nc) as tc:
        with (
            tc.tile_pool(name="const", bufs=1) as cpool,
            tc.tile_pool(name="io", bufs=1) as iopool,
            tc.tile_pool(name="pt", bufs=3) as ppool,
            tc.tile_pool(name="st", bufs=2, space="PSUM") as stpool,
            tc.tile_pool(name="ot", bufs=2, space="PSUM") as otpool,
        ):
            # single 128x128 0/1 lower-triangular corner mask (keep qq >= kk)
            ctri = cpool.tile([128, KT], F16, name="ctri")
            nc.sync.dma_start(out=ctri, in_=mk_d[:, :])

            # warmup matmuls: ~4us of sustained matmul activity moves the PE
            # clock (HAM) 1.2 -> 2.4 GHz before real compute. Uses the ctri
            # tile (first DMA to land) so they start immediately, no memset.
            n_warm = int(os.environ.get("ATTN_WARM", "36"))
            if n_warm:
                wps = otpool.tile([128, KT], F32, name="warmps", tag="otp")
                for _ in range(n_warm):
                    mi = nc.tensor.matmul(
                        out=wps, lhsT=ctri, rhs=ctri,
                        start=True, stop=True,
                    )
                    _note(mi, "warm")

            head_ctx = []
            for h in range(H_PER):
                vas = iopool.tile([128, NKT * VW], F16, name=f"vas{h}")
                qkts = iopool.tile([2 * D, 2 * S], F16, name=f"qkts{h}")
                outs = iopool.tile([VW, S], F32, name=f"outs{h}")
                # q-block 0 only needs the first 512 columns of q/k and the
                # first 4 V k-tiles: stage those first so compute starts
                # while the bulk still streams in
                if h == 0:
                    nc.sync.dma_start(out=vas[:, 0:4 * VW], in_=va_d[h][:, 0:4 * VW])
                    nc.sync.dma_start(out=qkts[:, 0:QB], in_=qk_d[h][:, 0:QB])
                    nc.sync.dma_start(
                        out=qkts[:, S:S + QB], in_=qk_d[h][:, S:S + QB]
                    )
                    nc.sync.dma_start(
                        out=vas[:, 4 * VW:], in_=va_d[h][:, 4 * VW:]
                    )
                    nc.sync.dma_start(out=qkts[:, QB:S], in_=qk_d[h][:, QB:S])
                    nc.sync.dma_start(
                        out=qkts[:, S + QB:2 * S], in_=qk_d[h][:, S + QB:2 * S]
                    )
                else:
                    nc.sync.dma_start(out=vas, in_=va_d[h])
                    # split halves onto separate DMA queues
                    nc.sync.dma_start(out=qkts[:, 0:S], in_=qk_d[h][:, 0:S])
                    nc.sync.dma_start(
                        out=qkts[:, S:2 * S], in_=qk_d[h][:, S:2 * S]
                    )
                head_ctx.append((vas, qkts, outs))

            # flat chunk list over (head, q-block): chunks of <=3 k-tiles;
            # one 3-bank PSUM tile + one exp (ScalarE or VectorE) per chunk
            all_chunks = []
            for h in range(H_PER):
                for j in range(NQB):
                    n_kt = 4 * (j + 1)          # causal: k-tiles 0..4j+3
                    k0 = 0
                    while k0 < n_kt:
                        c = min(3, n_kt - k0)
                        if c == 3 and n_kt - k0 == 4:
                            c = 2    # [2,2] packs mm1 pairs better than [3,1]
                        all_chunks.append((h, j, k0, c, n_kt))
                        k0 += c

            # engine routing: diagonal chunks -> ScalarE (exact exp);
            # off-diagonal chunks balance ScalarE/VectorE busy-time with a
            # preference for alternation (keeps both engines concurrently
            # busy within the software pipeline).
            eng_ns = {"act": 0.0, "dve": 0.0}
            routing = []
            prev = "act"
            for (h, j, k0, clen, n_kt) in all_chunks:
                is_diag = (k0 + clen - 1) >= 4 * j
                fd = clen * QB
                act_cost = (fd + 222.0) / 1.2
                dve_cost = (fd + 490.0) / 0.96   # measured: 2108ns @ fd=1536
                if is_diag:
                    eng = "act"
                    # diagonal corners cost VectorE mask time
                    n_corner = sum(
                        1 for u in range(clen) if (k0 + u) - 4 * j >= 0
                    )
                    eng_ns["dve"] += n_corner * 260.0
                else:
                    eng = "dve" if prev == "act" else "act"
                    if eng_ns[eng] > eng_ns["act" if eng == "dve" else "dve"] + 3000.0:
                        eng = "act" if eng == "dve" else "dve"
                eng_ns[eng] += act_cost if eng == "act" else dve_cost
                if k0 + clen == n_kt:
                    eng_ns["dve"] += 754.0       # PSUM->SBUF out copy
                routing.append(eng)
                prev = eng

            otp_box = {}

            def emit_mm1s(idx):
                h, j, k0, clen, n_kt = all_chunks[idx]
                vas, qkts, outs = head_ctx[h]
                stp = stpool.tile([128, 3 * QB], F32, name="stp", tag="stp")
                # QK^T matmuls two-at-a-time in disjoint row groups
                # (rows 0-63 / 64-127 hold identical q,k data) so the PE
                # runs them concurrently. Diagonal tiles only produce
                # q-columns >= 128t (start=True bank-clear zeroes the rest).
                u = 0
                while u < clen:
                    for r in range(2 if u + 1 < clen else 1):
                        ki = k0 + u + r
                        t = ki - 4 * j
                        off = KT * t if t > 0 else 0
                        row = slice(r * D, (r + 1) * D)
                        mi = nc.tensor.matmul(
                            out=stp[:, (u + r) * QB + off:(u + r + 1) * QB],
                            lhsT=qkts[row, S + ki * KT:S + (ki + 1) * KT],
                            rhs=qkts[row, j * QB + off:(j + 1) * QB],
                            start=True,
                            stop=True,
                        )
                        _note(mi, "qk_diag" if t > 0 else "qk")
                    u += 2 if u + 1 < clen else 1
                pt = ppool.tile([128, 3 * QB], F16, name="pt", tag="pt")
                # valid (written) column runs: diagonal tiles only produced
                # q-columns >= 128t, so merge per-tile valid ranges into
                # contiguous runs and exp only those (PSUM outside them is
                # uninitialized)
                runs = []
                for u in range(clen):
                    t = (k0 + u) - 4 * j
                    off = KT * t if t > 0 else 0
                    lo, hi = u * QB + off, (u + 1) * QB
                    if runs and runs[-1][1] == lo:
                        runs[-1][1] = hi
                    else:
                        runs.append([lo, hi])
                for lo, hi in runs:
                    if routing[idx] == "act":
                        nc.scalar.activation(
                            out=pt[:, lo:hi], in_=stp[:, lo:hi],
                            func=mybir.ActivationFunctionType.Exp,
                            scale=1.0 / EXP_A,
                        )
                    else:
                        nc.vector.tensor_scalar(
                            out=pt[:, lo:hi].bitcast(I16),
                            in0=stp[:, lo:hi],
                            scalar1=SCHRAUDOLPH_B,
                            scalar2=None,
                            op0=mybir.AluOpType.add,
                        )
                return pt

            def emit_mm2s(idx, pt):
                h, j, k0, clen, n_kt = all_chunks[idx]
                vas, qkts, outs = head_ctx[h]
                if (h, j) not in otp_box:
                    otp_box[(h, j)] = otpool.tile(
                        [VW, QB], F32, name="otp", tag="otp"
                    )
                otp = otp_box[(h, j)]
                for u in range(clen):
                    ki = k0 + u
                    t = ki - 4 * j
                    if t >= 0:
                        # in-place 128x128 triangular corner mask
                        cs = u * QB + KT * t
                        nc.vector.tensor_mul(
                            out=pt[:, cs:cs + KT],
                            in0=pt[:, cs:cs + KT],
                            in1=ctri,
                        )
                        off = KT * t
                    else:
                        off = 0
                    mi = nc.tensor.matmul(
                        out=otp[:, off:QB],
                        lhsT=vas[:, ki * VW:(ki + 1) * VW],
                        rhs=pt[:, u * QB + off:(u + 1) * QB],
                        start=(ki == 0),
                        stop=(ki == n_kt - 1),
                    )
                    _note(mi, "pv_diag" if t >= 0 else "pv")
                if k0 + clen == n_kt:       # last chunk of this q-block
                    nc.vector.tensor_copy(
                        out=outs[:, j * QB:(j + 1) * QB], in_=otp
                    )
                    nc.sync.dma_start(
                        out=oT_d[h][:, j * QB:(j + 1) * QB],
                        in_=outs[:, j * QB:(j + 1) * QB],
                    )

            # 1-deep software pipeline: emit the next chunk's QK matmuls and
            # exp before the current chunk's PV matmuls, so the exp engines
            # are never starved at q-block boundaries
            pending = None
            for idx in range(len(all_chunks)):
                pt = emit_mm1s(idx)
                if pending is not None:
                    emit_mm2s(*pending)
                pending = (idx, pt)
            emit_mm2s(*pending)

    # TRN2 allows at most 1 semaphore wait per instruction (the fp32r
    # matmul's LDWEIGHTS slot enforces it); split surplus waits into
    # standalone EventSemaphore instructions like the bacc flow does.
    import concourse.bacc as baccmod

    baccmod._bass_rust.generate_event_semaphores(nc)
    return nc


_PROGRAM_CACHE: dict[str, bass.Bass] = {}


def get_program() -> bass.Bass:
    if "p" not in _PROGRAM_CACHE:
        _PROGRAM_CACHE["p"] = build_program()
    return _PROGRAM_CACHE["p"]


def make_corner_mask() -> np.ndarray:
    kk = np.arange(128)[:, None]
    qq = np.arange(KT)[None, :]
    return np.ascontiguousarray((qq >= kk).astype(np.float16))


def make_in_maps(q, k, v):
    q = np.asarray(q, dtype=np.float32)
    k = np.asarray(k, dtype=np.float32)
    v = np.asarray(v, dtype=np.float32)
    mk = make_corner_mask()
    in_maps = []
    for c in range(N_CORES):
        hs = [H_PER * c + i for i in range(H_PER)]
        qk = np.empty((H_PER, 2 * D, 2 * S), dtype=np.float16)
        va = np.empty((H_PER, 128, NKT, VW), dtype=np.float16)
        for i, h in enumerate(hs):
            qk[i, 0:D, 0:S] = q[0, h].T * QK_SIDE_SCALE
            qk[i, 0:D, S:2 * S] = k[0, h].T * QK_SIDE_SCALE
            qk[i, D:2 * D, :] = qk[i, 0:D, :]
            # [S, D] -> k-tiles on partitions: [128, NKT, D]
            va[i, :, :, :D] = v[0, h].reshape(NKT, KT, D).transpose(1, 0, 2)
            va[i, :, :, D] = 1.0
        in_maps.append(
            {
                "qk": qk,
                "va": np.ascontiguousarray(va.reshape(H_PER, 128, NKT * VW)),
                "mk": mk,
            }
        )
    return in_maps


def assemble_output(results) -> np.ndarray:
    out = np.empty((B, H, S, D), dtype=np.float32)
    for c in range(N_CORES):
        oT = results[c]["outT"]  # [H_PER, VW, S]
        for i in range(H_PER):
            h = H_PER * c + i
            out[0, h] = (oT[i, :D, :] / oT[i, D:D + 1, :]).T
    return out


def run_sharded(q, k, v, trace: bool = False):
    from concourse.bass_utils import run_bass_kernel_spmd

    nc = get_program()
    in_maps = make_in_maps(q, k, v)
    res = run_bass_kernel_spmd(
        nc, in_maps, list(range(N_CORES)), trace=trace
    )
    return assemble_output(res.results), res


def kernel(q, k, v, mask=None) -> np.ndarray:
    # mask is deterministically the causal tril mask; causality is baked in.
    out, _ = run_sharded(q, k, v, trace=False)
    return out
